# revision 36
# baseline (speedup 1.0000x reference)
"""MoE layer (B=4,S=2048,D=1024,I=4096,E=8,top_k=2) on 8 TRN2 NeuronCores.

Strategy: expert-parallel over the FFN hidden (I) axis, perfectly balanced.
 - Host: router matmul (tiny), top-k + softmax gates, group tokens by expert.
 - Every core processes ALL routed token-slots (sum of expert counts =
   T*top_k = 16384) but only a 512-wide slice of I: core c holds
   W1[e][:, 512c:512(c+1)] and W2[e][512c:512(c+1), :] for every expert e.
   Per-core work is exactly total/8 regardless of routing imbalance, and the
   instruction stream is identical on all cores (pure SPMD; only weight DATA
   differs), unlike expert-per-core which pays the max expert count.
 - Device per (expert, token-chunk): h = gelu(xT @ W1slice + b1slice);
   y_partial = hT' @ W2slice, written to DRAM in bf16.
 - Host: sum the 8 partial y's, scale by gates, add b2, scatter-add.

All DRAM<->SBUF transfers are host-packed to the exact SBUF layout so each
is a single fully-dense [128, N] DMA (one ~600ns Sync trigger each instead
of 8-32): x is 1 trigger/chunk, y 1 trigger/chunk, weights 2KB-row slabs.
"""

import os

import ml_dtypes
import numpy as np

import concourse.bass as bass
import concourse.bacc as bacc
import concourse.mybir as mybir
import concourse.tile as tile
from concourse.bass_utils import run_bass_kernel_spmd

BF16 = mybir.dt.bfloat16
F32 = mybir.dt.float32
P = 128
N_CORES = 8
ISLICE = 512  # per-core I columns

# Filled with the profiled exec time (ns) of the last run when
# BASS_KERNEL_TRACE=1 is set in the environment (used by test.py).
LAST_EXEC_NS = None
LAST_RESULTS = None

_cache: dict = {}


def _chunks_for(count: int) -> list[int]:
    """Split a token count into matmul free-dim chunks <=512 (PSUM bank
    limit). Tails <192 are merged with the previous 512 and split evenly
    so no chunk is narrow enough for LDWEIGHTS to dominate."""
    if count == 0:
        return []
    full, rem = divmod(count, 512)
    if rem == 0:
        return [512] * full
    if rem >= 192 or full == 0:
        return [512] * full + [rem]
    tot = 512 + rem
    return [512] * (full - 1) + [(tot + 1) // 2, tot // 2]


def _build(chunk_plan: tuple[tuple[int, tuple[int, ...]], ...], D: int, I: int):
    """One-core program: for each (expert, token chunk) in processing
    order, FFN on this core's I-slice. chunk_plan = ((expert_id, chunk
    widths), ...) — experts ordered so the kernel ends with the smallest
    tail chunk (cheap final y writeback)."""
    KD = D // P  # 8  k-tiles for m1 contraction over D
    KI = ISLICE // P  # 4  k-tiles for m2 contraction over the I slice
    ND = D // P  # 8  output d-tiles
    E = len(chunk_plan)
    tot_slots = sum(sum(c) for _, c in chunk_plan)

    nc = bacc.Bacc()
    # Host-packed layouts (per partition p, contiguous within a row):
    #  xp : per chunk slab [k(KD), c(cw)]                -> [P, KD*tot_slots]
    #  w1 : per (e, j) slab [k(KD), i(P)]                -> [P, E*KI*KD*P]
    #  w2 : per (e, k) slab [d(D)]                       -> [P, E*KI*D]
    #  b1 : [e, j]                                       -> [P, E*KI]
    #  yp : per chunk slab [d(ND), c(cw)]                -> [P, ND*tot_slots]
    xp = nc.declare_dram_parameter("xp", [P, KD * tot_slots], BF16, isOutput=False)
    w1 = nc.declare_dram_parameter("w1", [P, E * KI * KD * P], BF16, isOutput=False)
    w2 = nc.declare_dram_parameter("w2", [P, E * KI * D], BF16, isOutput=False)
    b1 = nc.declare_dram_parameter("b1", [P, E * KI], F32, isOutput=False)
    yp = nc.declare_dram_parameter("yp", [P, ND * tot_slots], BF16, isOutput=True)

    with tile.TileContext(nc) as tc:
        with (
            tc.tile_pool(name="wpool", bufs=1) as wpool,
            tc.tile_pool(name="cpool", bufs=1) as cpool,
            tc.tile_pool(name="xpool", bufs=3) as xpool,
            tc.tile_pool(name="hpool", bufs=3) as hpool,
            tc.tile_pool(name="ypool", bufs=2) as ypool,
            tc.tile_pool(name="pspool", bufs=8, space="PSUM") as pspool,
        ):
            # The 16 DMA engines fair-share packets of all in-flight
            # transfers (~280GB/s/core aggregate, ~3us trigger->first
            # packet). The first real matmul is gated on chunk-0's x slab
            # plus the first expert's first w1 slab, so those triggers go
            # first, in consumption order.
            w1_sb = [[None] * KI for _ in range(E)]
            w2_sb = [[None] * KI for _ in range(E)]

            def _load_w1(e, js):
                for j in js:
                    t = wpool.tile([P, KD * P], BF16, tag=f"w1_{e}_{j}")
                    off = (e * KI + j) * KD * P
                    nc.sync.dma_start(out=t[:], in_=w1[:, off : off + KD * P])
                    w1_sb[e][j] = t

            def _load_w2(e, ks=None):
                for k in ks if ks is not None else range(KI):
                    t = wpool.tile([P, D], BF16, tag=f"w2_{e}_{k}")
                    off = (e * KI + k) * D
                    nc.sync.dma_start(out=t[:], in_=w2[:, off : off + D])
                    w2_sb[e][k] = t

            def _load_w(e):
                _load_w1(e, range(KI))
                _load_w2(e)

            chunks = []  # (expert, cw, slot_offset) in processing order
            off = 0
            for e, widths in chunk_plan:
                for cw in widths:
                    chunks.append((e, cw, off))
                    off += cw

            x_tiles = {}

            def _load_x(ci):
                if ci >= len(chunks) or ci in x_tiles:
                    return
                _, cw, soff = chunks[ci]
                t = xpool.tile([P, KD * cw], BF16, tag="x")
                nc.sync.dma_start(
                    out=t[:], in_=xp[:, KD * soff : KD * (soff + cw)]
                )
                x_tiles[ci] = t

            e0 = chunks[0][0]
            _load_w1(e0, [0])
            _load_x(0)
            b1_sb = cpool.tile([P, E * KI], F32, tag="b1")
            nc.sync.dma_start(out=b1_sb[:], in_=b1[:])
            _load_w1(e0, [1, 2, 3])
            _load_w2(e0, [0, 1])
            _load_x(1)
            _load_w2(e0, [2, 3])

            # ACT warm-up: absorb the b1 DMA into ACT's vector clock once so
            # real gelus only need their PSUM RAW wait.
            warm = cpool.tile([1, 1], F32, tag="warm")
            warm2 = cpool.tile([1, 1], F32, tag="warm2")
            nc.scalar.copy(warm[:], b1_sb[:1, :1])

            # HAM warm-up: dummy matmuls on zeroed scratch while the first
            # x/w1 DMAs stream, so real matmuls start at 2.4 GHz. 14 cold
            # dummies (~6us) end right when chunk-0's x lands, covering the
            # 3.4us busy window needed to unthrottle with no PE idle gap.
            scratch = cpool.tile([P, 512], BF16, tag="scratch")
            nc.gpsimd.memset(scratch[:], 0.0)
            for _ in range(2):
                pw = pspool.tile([P, 512], F32, tag="ps")
                for k in range(7):
                    nc.tensor.matmul(
                        pw[:],
                        scratch[:, :P],
                        scratch[:],
                        start=(k == 0),
                        stop=(k == 6),
                    )

            order = list(dict.fromkeys(e for e, _, _ in chunks))
            oi_of = {e: i for i, e in enumerate(order)}
            nchunks_of = {e: sum(1 for ee, _, _ in chunks if ee == e) for e in order}
            state = {"next_wi": 1, "ci_in_e": -1, "prev_e": None, "prev_h": None}
            h_of = {}

            def emit_m1(ci):
                e, cw, soff = chunks[ci]
                state["ci_in_e"] = (
                    state["ci_in_e"] + 1 if e == state["prev_e"] else 0
                )
                state["prev_e"] = e
                _load_x(ci + 1)
                x_sb = x_tiles.pop(ci)
                # Load the next expert's weights during the current
                # expert's 2nd chunk (so x prefetch triggers stay ahead of
                # the weight bulk in the DMA queues).
                if (
                    state["next_wi"] < len(order)
                    and state["next_wi"] == oi_of[e] + 1
                    and (state["ci_in_e"] >= 1 or nchunks_of[e] == 1)
                ):
                    _load_w(order[state["next_wi"]])
                    state["next_wi"] += 1
                if state["prev_h"] is not None:
                    # Advance ACT's observed self-tick past the previous
                    # chunk's gelus so h-tile WAW deps don't need a second
                    # sync wait per gelu.
                    nc.scalar.copy(warm[:], state["prev_h"][:1, :1])
                    nc.scalar.copy(warm2[:], warm[:])
                # m1: hT[j] = gelu(W1slice_j.T @ x + b1), j over KI I-tiles
                h_sb = []
                for j in range(KI):
                    ps = pspool.tile([P, cw], F32, tag="ps")
                    for k in range(KD):
                        nc.tensor.matmul(
                            ps[:],
                            w1_sb[e][j][:, k * P : (k + 1) * P],
                            x_sb[:, k * cw : (k + 1) * cw],
                            start=(k == 0),
                            stop=(k == KD - 1),
                        )
                    ht = hpool.tile([P, cw], BF16, tag=f"h_{j}")
                    nc.scalar.activation(
                        ht[:],
                        ps[:],
                        mybir.ActivationFunctionType.Gelu,
                        bias=b1_sb[:, e * KI + j : e * KI + j + 1],
                    )
                    h_sb.append(ht)
                state["prev_h"] = h_sb[-1]
                h_of[ci] = h_sb

            def emit_m2(ci):
                e, cw, soff = chunks[ci]
                h_sb = h_of.pop(ci)
                # m2: y[d] = sum_k W2slice_k[:, d].T @ h[k]  (partial over I)
                # y is written in groups of d-tiles whose DMAs start while
                # later d-tiles' matmuls still run, so only the final group
                # of y writeback is exposed after the last matmul. The very
                # last chunk uses 2-d-tile groups to shrink that exposure.
                NG = ND // 2 if ci < len(chunks) - 1 else ND // 4
                ngroups = ND // NG
                y_g = [
                    ypool.tile(
                        [P, NG * cw], BF16, tag=f"y{g % 2}", name=f"y_g{g}"
                    )
                    for g in range(ngroups)
                ]
                for dd in range(ND):
                    ps = pspool.tile([P, cw], F32, tag="ps")
                    for k in range(KI):
                        nc.tensor.matmul(
                            ps[:],
                            w2_sb[e][k][:, dd * P : (dd + 1) * P],
                            h_sb[k][:],
                            start=(k == 0),
                            stop=(k == KI - 1),
                        )
                    g, gi = divmod(dd, NG)
                    nc.vector.tensor_copy(
                        y_g[g][:, gi * cw : (gi + 1) * cw], ps[:]
                    )
                    if gi == NG - 1:
                        nc.sync.dma_start(
                            out=yp[
                                :,
                                ND * soff + g * NG * cw : ND * soff
                                + (g + 1) * NG * cw,
                            ],
                            in_=y_g[g][:],
                        )

            # The first two chunks run m1 before any m2: the first m2
            # needs all of w2[e0] in SBUF, and under the 8-core startup
            # HBM crunch (~280GB/s/core) that lands ~4us after w1[e0]+x0.
            # Back-to-back m1(c0), m1(c1) keeps the PE busy until it does.
            # (Depth 3 measured WORSE: it pushes chunk-2's x behind w2 in
            # the DMA queues and starves m1(c2) for ~4us.)
            if len(chunks) >= 3:
                emit_m1(0)
                emit_m1(1)
                emit_m2(0)
                emit_m2(1)
                rest = range(2, len(chunks))
            else:
                rest = range(len(chunks))
            for ci in rest:
                emit_m1(ci)
                emit_m2(ci)
    nc.compile()
    return nc, chunks, tot_slots


def kernel(**inputs) -> np.ndarray:
    global LAST_EXEC_NS, LAST_RESULTS
    x = np.asarray(inputs["x"], dtype=np.float32)
    Wr = np.asarray(inputs["Wr"], dtype=np.float32)
    br = np.asarray(inputs["br"], dtype=np.float32)
    W1 = np.asarray(inputs["W1"], dtype=np.float32)
    b1 = np.asarray(inputs["b1"], dtype=np.float32)
    W2 = np.asarray(inputs["W2"], dtype=np.float32)
    b2 = np.asarray(inputs["b2"], dtype=np.float32)
    K = int(np.asarray(inputs["top_k"]))

    B, S, D = x.shape
    E = Wr.shape[0]
    I = W1.shape[2]
    T = B * S
    KD = D // P
    KI = ISLICE // P
    ND = D // P
    xf = x.reshape(T, D)

    # Router (tiny) on host: logits -> top-k (desc, ties -> lower index,
    # matching jax.lax.top_k) -> softmax over the selected k.
    logits = xf @ Wr.T + br
    order = np.argsort(-logits, axis=-1, kind="stable")[:, :K]
    topv = np.take_along_axis(logits, order, axis=-1)
    exv = np.exp(topv - topv.max(axis=-1, keepdims=True))
    gates = (exv / exv.sum(axis=-1, keepdims=True)).astype(np.float32)

    toks, gvals = [], []
    for e in range(E):
        sel = order == e
        tok = np.nonzero(sel.any(axis=-1))[0]
        kidx = np.argmax(sel[tok], axis=-1)
        toks.append(tok)
        gvals.append(gates[tok, kidx].astype(np.float32))

    # Experts ordered by descending tail-chunk width: the kernel then ends
    # on the smallest chunk, minimizing the exposed final y writeback. The
    # first expert additionally starts with a 128-wide chunk.
    tails = [(_chunks_for(len(t)) or [0])[-1] for t in toks]
    expert_order = sorted(range(E), key=lambda e: -tails[e])
    plan = [list(_chunks_for(len(toks[e]))) for e in expert_order]
    # End the kernel on a ~128-wide chunk so the final exposed y writeback
    # (after the last matmul) is small.
    if plan and plan[-1] and plan[-1][-1] > 256:
        c = plan[-1].pop()
        plan[-1] += [c - 128, 128]
    chunk_plan = tuple(
        (e, tuple(p)) for e, p in zip(expert_order, plan)
    )
    key = (chunk_plan, D, I)
    if key not in _cache:
        _cache[key] = _build(chunk_plan, D, I)
    nc, chunks, tot_slots = _cache[key]

    bf = ml_dtypes.bfloat16
    # Pack x once: [P, KD*tot_slots], per chunk slab [k, c] within a row.
    xp = np.empty((P, KD * tot_slots), dtype=bf)
    for e in range(E):
        n = len(toks[e])
        if n == 0:
            continue
        # [n, D] -> [D, n] -> [KD, P, n]
        xe = np.ascontiguousarray(xf[toks[e]].T.astype(bf)).reshape(KD, P, n)
        off = 0
        for ce, cw, soff in chunks:
            if ce != e:
                continue
            # slab [P, KD, cw]
            xp[:, KD * soff : KD * (soff + cw)] = (
                xe[:, :, off : off + cw].transpose(1, 0, 2).reshape(P, KD * cw)
            )
            off += cw

    in_maps = []
    for c in range(N_CORES):
        i0 = c * ISLICE
        # w1 packed: [P, E*KI*KD*P]; slab (e, j) = [k, i] within a row,
        # element (p, e, j, k, i) = W1[e][k*P + p, i0 + j*P + i]
        w1c = (
            W1[:, :, i0 : i0 + ISLICE]
            .reshape(E, KD, P, KI, P)
            .transpose(2, 0, 3, 1, 4)  # p, e, j, k, i
            .reshape(P, E * KI * KD * P)
            .astype(bf)
        )
        # w2 packed: [P, E*KI*D]; slab (e, k) = [d] within a row,
        # element (p, e, k, d) = W2[e][i0 + k*P + p, d]
        w2c = (
            W2[:, i0 : i0 + ISLICE, :]
            .reshape(E, KI, P, D)
            .transpose(2, 0, 1, 3)  # p, e, k, d
            .reshape(P, E * KI * D)
            .astype(bf)
        )
        # b1 packed: [P, E*KI]: element (p, e, j) = b1[e][i0 + j*P + p]
        b1c = np.ascontiguousarray(
            b1[:, i0 : i0 + ISLICE].reshape(E, KI, P).transpose(2, 0, 1).reshape(P, E * KI)
        )
        in_maps.append(
            {
                "xp": xp if c == 0 else xp.copy(),
                "w1": np.ascontiguousarray(w1c),
                "w2": np.ascontiguousarray(w2c),
                "b1": b1c,
            }
        )

    trace = bool(int(os.environ.get("BASS_KERNEL_TRACE", "0")))
    if trace:
        try:
            from antenv.axon_hooks import get_axon_ntff_profile_hook  # noqa: F401
        except ImportError:
            trace = False
    res = run_bass_kernel_spmd(
        nc, in_maps, core_ids=list(range(N_CORES)), trace=trace
    )
    LAST_EXEC_NS = res.exec_time_ns
    LAST_RESULTS = res

    # Sum the 8 I-slice partials, then scatter-add gate * (y + b2).
    ysum = np.zeros((P, ND * tot_slots), dtype=np.float32)
    for c in range(N_CORES):
        ysum += res.results[c]["yp"].astype(np.float32)

    out = np.zeros((T, D), dtype=np.float32)
    for e in range(E):
        n = len(toks[e])
        if n == 0:
            continue
        ye = np.empty((n, D), dtype=np.float32)
        off = 0
        for ce, cw, soff in chunks:
            if ce != e:
                continue
            slab = ysum[:, ND * soff : ND * (soff + cw)].reshape(P, ND, cw)
            # y[d_tile*P + p, c]
            ye[off : off + cw] = slab.transpose(2, 1, 0).reshape(cw, D)
            off += cw
        out[toks[e]] += gvals[e][:, None] * (ye + b2[e][None, :])
    return out.reshape(B, S, D)
